# revision 39
# baseline (speedup 1.0000x reference)
"""Multi-Head Latent Attention (MLA) Trainium2 kernel, 8-core head-sharded,
with cross-core sharding of the latent down-projections.

v2 structural change vs baseline: stage 1 (c_Q / c_KV / k_R latents) was
replicated on all 8 cores (52% of PE cycles). Now:
  - query block 0's latents stay replicated (they are needed first, and
    the first collective on this fabric cannot complete before ~75us due
    to rank-launch skew - the replicated compute fills that window);
  - query blocks 1-3's latents are token-sharded 8 ways: each core
    computes all 9 latent chains for its own 64-token slice of each
    block (one fused 192-column matmul per (m-chunk, chain)), ropes its
    k_R slice locally, and three AllGathers (one per block, ~0.55MB wire
    per rank each) distribute the full latents while block-0/1 attention
    runs on the PE.
Everything downstream (per-head projections, attention, output
projection) is unchanged from the baseline except q_R: both heads'
rope-projections now run packed in one 128-wide stationary matmul.

Layout: all matmuls run with the contraction dim on partitions
("transposed world"); x and every weight are pre-transposed on the host.
Heads are sharded 2-per-core; each core emits a bf16 partial out.T (its
heads' contribution to the output projection), summed and transposed on
the host (rel err ~4e-3, harness gate 2e-2).

Precision: bf16 operands everywhere on the PE; all PSUM accumulation is
fp32. k_R/q_R are zero-padded to 128 partitions, and W_KR columns are
host-duplicated so the kr matmul has a full 128-wide stationary. The
softmax denominator is a matmul against an all-ones [128,128] stationary;
the reciprocal (reciprocal_approx_fast) is multiplied in directly.
"""
import sys

sys.path.insert(0, "/opt/trn_rl_repo")

import numpy as np

import concourse.bass as bass
import concourse.tile as tile
from concourse import bacc, mybir
from concourse.bass_utils import run_bass_kernel_spmd

F32 = mybir.dt.float32
BF16 = mybir.dt.bfloat16
AF = mybir.ActivationFunctionType
OP = mybir.AluOpType

N_CORES = 8
S = 2048          # sequence length
DM = 2048         # d_model
DL = 512          # d_latent
H = 16            # total heads
HC = H // N_CORES  # heads per core (2)
DH = 128          # head dim (content)
DHR = 64          # head dim (rope)
QB = 512          # query block
NQB = S // QB     # 4
KPB = QB // 128   # key chunks per query block (4)
NMC = DM // 128   # 16 model chunks
NLC = DL // 128   # 4 latent chunks
NKC = S // 128    # 16 key chunks
SQB = QB // N_CORES  # sharded tokens per (rank, block) = 64
NSH = NQB - 1     # sharded (gathered) query blocks: 1..3
LAT = 9 * 128     # gathered latent rows per token-slice (kr padded to 128)
THETA = 10000.0

SCALE = float(1.0 / np.sqrt(np.float32(DH + DHR)))
E_HI = float(np.exp(np.float64(80.0) * SCALE))
E_LO = float(np.exp(np.float64(-80.0) * SCALE))

# Set by test.py to profile; harness path leaves these untouched.
TRACE = False
TRACE_KWARGS = {}
LAST_EXEC_TIME_NS = None
LAST_RESULTS = None
DEBUG_DUMP = False

_CACHE = {}
MM_LABELS = {}


def _lbl(inst, label):
    try:
        MM_LABELS[inst.ins.name] = label
    except Exception:
        try:
            MM_LABELS[inst.name] = label
        except Exception:
            pass
    return inst


def _build():
    nc = bacc.Bacc("TRN2", target_bir_lowering=False, debug=False,
                   enable_asserts=True, num_devices=N_CORES)

    def din(name, shape, dt=BF16):
        return nc.dram_tensor(name, shape, dt, kind="ExternalInput").ap()

    d = {
        "xT0": din("xT0", [DM, QB]),              # block-0 tokens (replicated)
        "xTs": din("xTs", [DM, NSH * SQB]),       # this core's token slices
        "wdqT": din("wdqT", [DM, DL]),
        "wdkvT": din("wdkvT", [DM, DL]),
        "wkrT": din("wkrT", [DM, 128]),
        "wuqT": din("wuqT", [DL, HC * DH]),
        "wqrT": din("wqrT", [DL, HC * DHR]),
        "wukT": din("wukT", [DL, HC * DH]),
        "wuvT": din("wuvT", [DL, HC * DH]),
        "woT": din("woT", [HC * DH, DM]),
        "ones128": din("ones128", [128, 128]),
        "masktri": din("masktri", [128, 128], F32),
        "zeros64": din("zeros64", [64, S]),
        "cs1": din("cs1", [DHR, S], F32),
        "cs2": din("cs2", [DHR, S], F32),
        "csk1": din("csk1", [DHR, NSH * SQB], F32),
        "csk2": din("csk2", [DHR, NSH * SQB], F32),
        "outT": nc.dram_tensor("outT", [DM, S], BF16,
                               kind="ExternalOutput").ap(),
    }
    with tile.TileContext(nc) as tc:
        import contextlib
        with contextlib.ExitStack() as ctx:
            _kernel_body(ctx, tc, nc, d)
    nc.compile()
    return nc


def _kernel_body(ctx, tc, nc, d):
    wts = ctx.enter_context(tc.tile_pool(name="wts", bufs=1))
    kvp = ctx.enter_context(tc.tile_pool(name="kvp", bufs=1))
    xtp = ctx.enter_context(tc.tile_pool(name="xtp", bufs=1))
    lat = ctx.enter_context(tc.tile_pool(name="lat", bufs=1))
    prj = ctx.enter_context(tc.tile_pool(name="prj", bufs=1))
    smp = ctx.enter_context(tc.tile_pool(name="smp", bufs=1))
    stg = ctx.enter_context(tc.tile_pool(name="stg", bufs=1))
    o5p = ctx.enter_context(tc.tile_pool(name="o5p", bufs=16))
    dram = ctx.enter_context(tc.tile_pool(name="dram", bufs=1, space="DRAM"))

    # ---- stage-1 weights first: the sharded m-loop consumes
    # wkr[m]+wdq[m]+wdkv[m] per m, so issue them interleaved per m across
    # two queues ----
    wkr_t = [wts.tile([128, 128], BF16, name=f"wkr{m}") for m in range(NMC)]
    wdq_t = [wts.tile([128, DL], BF16, name=f"wdq{m}") for m in range(NMC)]
    wdkv_t = [wts.tile([128, DL], BF16, name=f"wdkv{m}") for m in range(NMC)]
    # pass-0 weights (wkr+wdq) for all m first, then pass-1 (wdkv): the
    # sharded m-loop never starves on late wdkv chunks
    for m in range(NMC):
        e = nc.gpsimd if m % 2 == 0 else nc.scalar
        e.dma_start(wkr_t[m][:], d["wkrT"][m * 128:(m + 1) * 128, :])
        e.dma_start(wdq_t[m][:], d["wdqT"][m * 128:(m + 1) * 128, :])
    for m in range(NMC):
        e = nc.gpsimd if m % 2 == 0 else nc.scalar
        e.dma_start(wdkv_t[m][:], d["wdkvT"][m * 128:(m + 1) * 128, :])

    # x for the sharded token slices (needed first), then block-0 x
    xts = [xtp.tile([128, NSH * SQB], BF16, name=f"xts{m}")
           for m in range(NMC)]
    for m in range(NMC):
        nc.sync.dma_start(xts[m][:], d["xTs"][m * 128:(m + 1) * 128, :])
    xt0 = [xtp.tile([128, QB], BF16, name=f"xt0_{m}") for m in range(NMC)]
    for m in range(NMC):
        nc.sync.dma_start(xt0[m][:], d["xT0"][m * 128:(m + 1) * 128, :])

    # small persistent loads
    o128_t = wts.tile([128, 128], BF16, name="o128")
    nc.scalar.dma_start(o128_t[:], d["ones128"][:, :])
    mask_t = wts.tile([128, 128], F32, name="masktri")
    nc.scalar.dma_start(mask_t[:], d["masktri"][:, :])
    csk1_t = smp.tile([DHR, NSH * SQB], F32, name="csk1t")
    csk2_t = smp.tile([DHR, NSH * SQB], F32, name="csk2t")
    nc.scalar.dma_start(csk1_t[:], d["csk1"][:, :])
    nc.scalar.dma_start(csk2_t[:], d["csk2"][:, :])
    wuq_t = [wts.tile([128, HC * DH], BF16, name=f"wuq{l}") for l in range(NLC)]
    wqr_t = [wts.tile([128, HC * DHR], BF16, name=f"wqr{l}") for l in range(NLC)]
    wuk_t = [wts.tile([128, HC * DH], BF16, name=f"wuk{l}") for l in range(NLC)]
    wuv_t = [wts.tile([128, HC * DH], BF16, name=f"wuv{l}") for l in range(NLC)]
    wo_t = [wts.tile([128, DM], BF16, name=f"wo{h}") for h in range(HC)]

    def emit_proj_dmas():
        for l in range(NLC):
            nc.gpsimd.dma_start(wuk_t[l][:], d["wukT"][l * 128:(l + 1) * 128, :])
            nc.gpsimd.dma_start(wuv_t[l][:], d["wuvT"][l * 128:(l + 1) * 128, :])
            nc.gpsimd.dma_start(wuq_t[l][:], d["wuqT"][l * 128:(l + 1) * 128, :])
            nc.gpsimd.dma_start(wqr_t[l][:], d["wqrT"][l * 128:(l + 1) * 128, :])

    def emit_wo_dmas():
        for h in range(HC):
            nc.gpsimd.dma_start(wo_t[h][:], d["woT"][h * 128:(h + 1) * 128, :])

    # ---- persistent per-sequence state ----
    kct = [kvp.tile([128, S], BF16, name=f"kct{h}") for h in range(HC)]
    # krt/qrt are zero-padded to 128 partitions: a 64-partition moving
    # operand runs matmuls at half rate.
    krt = kvp.tile([128, S], BF16, name="krt")
    nc.scalar.dma_start(krt[DHR:128, :], d["zeros64"][:, :])
    qrt = [kvp.tile([128, QB], BF16, name=f"qrt{h}") for h in range(HC)]
    for h in range(HC):
        nc.scalar.dma_start(qrt[h][DHR:128, :], d["zeros64"][:, 0:QB])
    vt = [kvp.tile([128, HC * DH], BF16, name=f"vt{k}") for k in range(NKC)]

    # ---- collective bounce buffers (internal DRAM) ----
    # Each block's gather is split: kv half (ckv chains + kr, 640 rows)
    # first, q half (cq chains, 512 rows) second - the k/v projections
    # can start as soon as the kv half lands.
    KVR = 5 * 128  # kv-half rows (ckv0-3 + kr)
    inb_kv = [dram.tile([KVR, SQB], BF16, name=f"inbkv{i}")
              for i in range(NSH)]
    inb_q = [dram.tile([DL, SQB], BF16, name=f"inbq{i}") for i in range(NSH)]
    outb_kv = [dram.tile([N_CORES * KVR, SQB], BF16, addr_space="Shared",
                         name=f"outbkv{i}") for i in range(NSH)]
    outb_q = [dram.tile([N_CORES * DL, SQB], BF16, addr_space="Shared",
                        name=f"outbq{i}") for i in range(NSH)]

    def rope(raw_pt, out_ap, cs1s, cs2s, tag, w=QB, swap_e=nc.sync):
        """raw_pt: PSUM [64, w] pre-rope rows; out_ap: bf16 dest [64, w].
        Swap-halves DMA reads PSUM directly. All ALU legs stay off
        gpsimd: after phase A that queue belongs to the collectives."""
        raw = smp.tile([DHR, w], F32, tag=f"rope_srcc{w}", name=f"rc_{tag}")
        nc.scalar.copy(raw[:], raw_pt)
        rsw = smp.tile([DHR, w], F32, tag=f"rope_swp{w}", name=f"rs_{tag}")
        swap_e.dma_start(rsw[0:32, :], raw[32:64, :])
        swap_e.dma_start(rsw[32:64, :], raw[0:32, :])
        rawm = smp.tile([DHR, w], F32, tag=f"rope_raw{w}", name=f"rr_{tag}")
        nc.vector.tensor_tensor(rawm[:], raw_pt, cs1s, op=OP.mult)
        nc.vector.tensor_tensor(rsw[:], rsw[:], cs2s, op=OP.mult)
        nc.vector.tensor_tensor(out_ap, rawm[:], rsw[:], op=OP.add)

    # ================= PHASE A: latents =================
    # chains: 0-3 cq[l], 4-7 ckv[l], 8 kr
    def sharded_stage1():
        """All 9 latent chains for this core's 64-token slices of blocks
        1..3, one fused 192-column matmul per (m, chain). Two passes of
        <=5 chains: interleaving two accumulation chains in one PSUM
        bank corrupts both, so every chain gets its own bank. Staged
        into a single SBUF tile so each block's gather input is ONE
        DMA."""
        W = NSH * SQB

        def st_for(j, m):
            if j < 4:
                return wdq_t[m][:, j * 128:(j + 1) * 128], "s1s_cq"
            if j < 8:
                return wdkv_t[m][:, (j - 4) * 128:(j - 3) * 128], "s1s_ckv"
            return wkr_t[m][:], "s1s_kr"

        ps = [tc.tile([128, W], F32, space="PSUM", name=f"pslat{t}")
              for t in range(5)]
        # staging layout: cq chains at cols 0:4W, ckv at 4W:8W, kr 8W:9W
        stall = stg.tile([128, 9 * W], BF16, name="stall")
        bank = {0: 0, 1: 1, 2: 2, 3: 3, 8: 4, 4: 0, 5: 1, 6: 2, 7: 3}
        for pa, chains in enumerate([(0, 1, 2, 3, 8), (4, 5, 6, 7)]):
            for m in range(NMC):
                for j in chains:
                    st_ap, label = st_for(j, m)
                    _lbl(nc.tensor.matmul(ps[bank[j]][0][:], st_ap,
                                          xts[m][:], start=(m == 0),
                                          stop=(m == NMC - 1)), label)
            for j in chains:
                if j == 8:
                    continue
                (nc.vector.tensor_copy if j % 2 == 0 else nc.scalar.copy)(
                    stall[:, j * W:(j + 1) * W], ps[bank[j]][0][:])
            if pa == 0:
                krp = ps[4][0][0:DHR, :]
                rope(krp, stall[0:DHR, 8 * W:9 * W], csk1_t[:], csk2_t[:],
                     "krs", w=W, swap_e=nc.scalar)
                # kr slot partitions 64-127 are never computed; zero them
                # so the gather doesn't ship uninitialized SBUF
                nc.scalar.copy(stall[DHR:128, 8 * W:9 * W], krt[DHR:128, 0:W])
        for i in range(NSH):
            isl = slice(i * SQB, (i + 1) * SQB)
            src_kv = stall[:, 4 * W:9 * W].rearrange(
                "p (j w) -> p j w", w=W)[:, :, isl]
            nc.gpsimd.dma_start(
                inb_kv[i][:].rearrange("(j p) t -> p j t", p=128), src_kv)
            src_q = stall[:, 0:4 * W].rearrange(
                "p (j w) -> p j w", w=W)[:, :, isl]
            nc.gpsimd.dma_start(
                inb_q[i][:].rearrange("(j p) t -> p j t", p=128), src_q)
        return ps

    def issue_ags():
        for i in range(NSH):
            nc.gpsimd.collective_compute(
                "AllGather", mybir.AluOpType.bypass,
                replica_groups=[list(range(N_CORES))],
                ins=[inb_kv[i].opt()], outs=[outb_kv[i].opt()])
            nc.gpsimd.collective_compute(
                "AllGather", mybir.AluOpType.bypass,
                replica_groups=[list(range(N_CORES))],
                ins=[inb_q[i].opt()], outs=[outb_q[i].opt()])

    srot = [0]

    def rep_stage1_qb0():
        """Block-0 latents replicated on every core (baseline stage-1)."""
        cs1s = smp.tile([DHR, QB], F32, tag="cs1s", bufs=2, name="cs1s0")
        cs2s = smp.tile([DHR, QB], F32, tag="cs2s", bufs=2, name="cs2s0")
        nc.scalar.dma_start(cs1s[:], d["cs1"][:, 0:QB])
        nc.scalar.dma_start(cs2s[:], d["cs2"][:, 0:QB])
        ckv = [lat.tile([128, QB], BF16, tag=f"ckv{l}", bufs=2,
                        name=f"ckv{l}_0") for l in range(NLC)]
        cq = [lat.tile([128, QB], BF16, tag=f"cq{l}", bufs=2,
                       name=f"cq{l}_0") for l in range(NLC)]
        eng_tgl = [0]

        def copy_out(dst, src):
            (nc.vector.tensor_copy if eng_tgl[0] % 2 == 0
             else nc.scalar.copy)(dst, src)
            eng_tgl[0] += 1

        plan = [
            [("kr", None), ("cq", 0), ("cq", 1)],
            [("cq", 2), ("cq", 3)],
            [("ckv", 0), ("ckv", 1)],
            [("ckv", 2), ("ckv", 3)],
        ]
        # 3 rotating PSUM banks across the passes (5 banks hold the
        # sharded chains until the gather inputs are staged)
        rep_ps = [tc.tile([128, QB], F32, space="PSUM", name=f"repps{i}")
                  for i in range(3)]
        rot = [0]
        for pi, groups in enumerate(plan):
            pts = []
            for gi, (kind, idx) in enumerate(groups):
                pts.append(rep_ps[rot[0] % 3][0])
                rot[0] += 1
            for m in range(NMC):
                for gi, (kind, idx) in enumerate(groups):
                    if kind == "kr":
                        st_ap, label = wkr_t[m][:], "s1_kr"
                    elif kind == "cq":
                        st_ap = wdq_t[m][:, idx * 128:(idx + 1) * 128]
                        label = "s1_cq"
                    else:
                        st_ap = wdkv_t[m][:, idx * 128:(idx + 1) * 128]
                        label = "s1_ckv"
                    _lbl(nc.tensor.matmul(pts[gi][:], st_ap, xt0[m][:],
                                          start=(m == 0),
                                          stop=(m == NMC - 1)), label)
            for gi, (kind, idx) in enumerate(groups):
                if kind == "kr":
                    rope(pts[gi][0:DHR, :], krt[0:DHR, 0:QB], cs1s[:],
                         cs2s[:], "kr0", swap_e=nc.scalar)
                elif kind == "cq":
                    copy_out(cq[idx][:], pts[gi][:])
                else:
                    copy_out(ckv[idx][:], pts[gi][:])
            if pi == 0:
                emit_proj_dmas()
                emit_wo_dmas()
        for t, free in reversed(rep_ps):
            free()
        return cq, ckv, cs1s, cs2s

    pslat = sharded_stage1()
    issue_ags()
    cq0, ckv0, cs1s0, cs2s0 = rep_stage1_qb0()
    for t, free in reversed(pslat):
        free()

    # ================= PHASE B: attention =================
    ps_s1 = ctx.enter_context(tc.tile_pool(name="ps_s1", bufs=1, space="PSUM"))
    ps_at = ctx.enter_context(tc.tile_pool(name="ps_at", bufs=2, space="PSUM"))
    ps_sm = ctx.enter_context(tc.tile_pool(name="ps_sm", bufs=2, space="PSUM"))

    s1rot = [0]

    def s1tile(shape, name):
        t = ps_s1.tile(shape, F32, tag=f"s1{s1rot[0] % 4}", name=name)
        s1rot[0] += 1
        return t

    def dma_back(qb):
        """Load block qb's gathered latents from the AllGather output.
        One 3D-access-pattern DMA per SBUF tile (9 per block): rank r's
        [128, 64] sub-block lands at columns r*64."""
        i = qb - 1
        ckv = [lat.tile([128, QB], BF16, tag=f"ckv{l}", bufs=2,
                        name=f"ckv{l}_{qb}") for l in range(NLC)]
        cq = [lat.tile([128, QB], BF16, tag=f"cq{l}", bufs=2,
                       name=f"cq{l}_{qb}") for l in range(NLC)]
        gkv = outb_kv[i][:].rearrange("(r q) t -> r q t", r=N_CORES)
        for l in range(NLC):
            nc.sync.dma_start(
                ckv[l][:].rearrange("p (r t) -> p r t", r=N_CORES),
                gkv[:, l * 128:(l + 1) * 128, :].transpose([1, 0, 2]))
        nc.sync.dma_start(
            krt[0:DHR, qb * QB:(qb + 1) * QB].rearrange(
                "p (r t) -> p r t", r=N_CORES),
            gkv[:, 4 * 128:4 * 128 + DHR, :].transpose([1, 0, 2]))
        gq = outb_q[i][:].rearrange("(r q) t -> r q t", r=N_CORES)
        for l in range(NLC):
            nc.sync.dma_start(
                cq[l][:].rearrange("p (r t) -> p r t", r=N_CORES),
                gq[:, l * 128:(l + 1) * 128, :].transpose([1, 0, 2]))
        cs1s = smp.tile([DHR, QB], F32, tag="cs1s", bufs=2, name=f"cs1s{qb}")
        cs2s = smp.tile([DHR, QB], F32, tag="cs2s", bufs=2, name=f"cs2s{qb}")
        qsl = slice(qb * QB, (qb + 1) * QB)
        nc.scalar.dma_start(cs1s[:], d["cs1"][:, qsl])
        nc.scalar.dma_start(cs2s[:], d["cs2"][:, qsl])
        return cq, ckv, cs1s, cs2s

    def stage2(qb, cq, ckv, cs1s, cs2s):
        # kv half first: ckv arrives one collective earlier than cq
        qsl = slice(qb * QB, (qb + 1) * QB)
        for h in range(HC):
            pkc = s1tile([128, QB], f"pkc{h}_{qb}")
            for l in range(NLC):
                _lbl(nc.tensor.matmul(pkc[:], wuk_t[l][:, h * DH:(h + 1) * DH],
                                      ckv[l][:], start=(l == 0),
                                      stop=(l == NLC - 1)), "s2_kc")
            (nc.vector.tensor_copy if h == 0 else nc.scalar.copy)(
                kct[h][:, qsl], pkc[:])
        for sc in range(KPB):
            k = qb * KPB + sc
            pv = s1tile([128, HC * DH], f"pvv{k}")
            for l in range(NLC):
                _lbl(nc.tensor.matmul(pv[:], ckv[l][:, sc * 128:(sc + 1) * 128],
                                      wuv_t[l][:], start=(l == 0),
                                      stop=(l == NLC - 1)), "s2_v")
            (nc.vector.tensor_copy if sc % 2 == 0 else nc.scalar.copy)(
                vt[k][:], pv[:])
        # q_R both heads packed in one 128-wide stationary chain
        pqr = s1tile([128, QB], f"pqr_{qb}")
        for l in range(NLC):
            _lbl(nc.tensor.matmul(pqr[:], wqr_t[l][:], cq[l][:],
                                  start=(l == 0), stop=(l == NLC - 1)),
                 "s2_qr")
        for h in range(HC):
            rope(pqr[h * DHR:(h + 1) * DHR, :], qrt[h][0:DHR, :],
                 cs1s[:], cs2s[:], f"qr{h}_{qb}")
        # q_C per head
        qct = [prj.tile([128, QB], BF16, tag=f"qct{h}", bufs=2,
                        name=f"qct{h}_{qb}") for h in range(HC)]
        for h in range(HC):
            pqc = s1tile([128, QB], f"pqc{h}_{qb}")
            for l in range(NLC):
                _lbl(nc.tensor.matmul(pqc[:], wuq_t[l][:, h * DH:(h + 1) * DH],
                                      cq[l][:], start=(l == 0),
                                      stop=(l == NLC - 1)), "s2_qc")
            nc.vector.tensor_copy(qct[h][:], pqc[:])
        return qct

    def attn_both(qb, qct, qrt):
        """Both heads interleaved per key chunk: 2x PE density per chain
        step."""
        nkc = KPB * (qb + 1)
        pat = [ps_at.tile([128, QB], F32, tag="at", name=f"pat{h}_{qb}")
               for h in range(HC)]
        psums = [ps_sm.tile([128, QB], F32, tag="smrb", name=f"psums{h}_{qb}")
                 for h in range(HC)]
        pend = []  # (h, kc, off, pt) awaiting PV+sums

        def flush(last):
            h, kc, off, pt = pend.pop(0)
            _lbl(nc.tensor.matmul(psums[h][:, off:], o128_t[:], pt[:, off:],
                                  start=(kc == 0), stop=last,
                                  skip_group_check=True), "sum")
            _lbl(nc.tensor.matmul(pat[h][:, off:],
                                  vt[kc][:, h * DH:(h + 1) * DH],
                                  pt[:, off:], start=(kc == 0), stop=last,
                                  skip_group_check=True), "pv")

        for kc in range(nkc):
            off = 128 * (kc - KPB * qb) if kc >= KPB * qb else 0
            w = QB - off
            ksl = slice(kc * 128, (kc + 1) * 128)
            for h in range(HC):
                ps_s = s1tile([128, QB], f"s{h}_{qb}_{kc}")
                _lbl(nc.tensor.matmul(ps_s[:, off:], kct[h][:, ksl],
                                      qct[h][:, off:], start=True, stop=False,
                                      skip_group_check=True), "qk_c")
                _lbl(nc.tensor.matmul(ps_s[:, off:], krt[:, ksl],
                                      qrt[h][:, off:], start=False, stop=True,
                                      skip_group_check=True), "qk_r")
                if len(pend) >= 2:
                    flush(False)
                et = smp.tile([128, QB], BF16, tag="et", bufs=5,
                              name=f"et{h}_{qb}_{kc}")
                nc.scalar.activation(et[:, off:], ps_s[:, off:], AF.Exp,
                                     scale=SCALE)
                pt = smp.tile([128, QB], BF16, tag="pt", bufs=5,
                              name=f"pt{h}_{qb}_{kc}")
                ce = nc.vector if (kc + h) % 2 == 0 else nc.gpsimd
                if kc >= KPB * qb:  # diagonal: clip+mask window, clip rest
                    ctw = smp.tile([128, 128], BF16, tag="ctw", bufs=3,
                                   name=f"ctw{h}_{qb}_{kc}")
                    ce.tensor_scalar(ctw[:], et[:, off:off + 128],
                                     E_HI, E_LO, op0=OP.min, op1=OP.max)
                    ce.tensor_tensor(pt[:, off:off + 128], ctw[:],
                                     mask_t[:], op=OP.mult)
                    if w > 128:
                        ce.tensor_scalar(pt[:, off + 128:],
                                         et[:, off + 128:], E_HI, E_LO,
                                         op0=OP.min, op1=OP.max)
                else:
                    ce.tensor_scalar(pt[:], et[:], E_HI, E_LO,
                                     op0=OP.min, op1=OP.max)
                pend.append((h, kc, off, pt))
        while len(pend) > 2:
            flush(False)
        while pend:
            flush(True)
        return pat, psums

    def attn_sum(qb, h, psums):  # psums: [128,QB], denom bcast on partitions
        rcr = smp.tile([128, QB], F32, tag="rcr", bufs=2, name=f"rcr{h}_{qb}")
        nc.vector.reciprocal_approx_fast(rcr[:], psums[:])
        return rcr

    def attn_norm(qb, h, pat, rcr, attn_n):
        nc.vector.tensor_tensor(attn_n[:], pat[:], rcr[:], op=OP.mult)

    def stage5(qb, attn_n):
        qsl = slice(qb * QB, (qb + 1) * QB)
        for m in range(NMC):
            po = s1tile([128, QB], f"po{m}_{qb}")
            for h in range(HC):
                _lbl(nc.tensor.matmul(po[:], wo_t[h][:, m * 128:(m + 1) * 128],
                                      attn_n[h][:], start=(h == 0),
                                      stop=(h == HC - 1)), "s5")
            ob = o5p.tile([128, QB], BF16, tag="ob", name=f"ob{m}_{qb}")
            (nc.vector.tensor_copy if m % 2 == 0 else nc.scalar.copy)(
                ob[:], po[:])
            nc.gpsimd.dma_start(d["outT"][m * 128:(m + 1) * 128, qsl], ob[:])

    # ---- main loop ----
    cq, ckv, cs1s, cs2s = cq0, ckv0, cs1s0, cs2s0
    qct = stage2(0, cq, ckv, cs1s, cs2s)
    for qb in range(NQB):
        attn_n = [prj.tile([128, QB], BF16, tag=f"an{h}", name=f"an{h}_{qb}")
                  for h in range(HC)]
        pat, psums = attn_both(qb, qct, qrt)
        rcr0 = attn_sum(qb, 0, psums[0])
        rcr1 = attn_sum(qb, 1, psums[1])
        attn_norm(qb, 0, pat[0], rcr0, attn_n[0][:])
        attn_norm(qb, 1, pat[1], rcr1, attn_n[1][:])
        if qb < NQB - 1:
            cq, ckv, cs1s, cs2s = dma_back(qb + 1)
            qct = stage2(qb + 1, cq, ckv, cs1s, cs2s)
        stage5(qb, attn_n)


def _prep_inputs(x, W_DQ, W_UQ, W_QR, W_DKV, W_UK, W_UV, W_KR, W_O):
    """Host-side sharding + layout prep. Returns list of 8 in_maps."""
    import ml_dtypes
    f32 = np.float32
    bf16 = ml_dtypes.bfloat16
    xT = np.ascontiguousarray(x[0].T).astype(bf16)
    perm = np.concatenate([np.arange(0, DHR, 2), np.arange(1, DHR, 2)])
    wdqT = np.ascontiguousarray(W_DQ.T).astype(bf16)
    wdkvT = np.ascontiguousarray(W_DKV.T).astype(bf16)
    wkrT = np.ascontiguousarray(
        np.concatenate([W_KR.T[:, perm]] * 2, axis=1)).astype(bf16)

    # rope tables (transposed, permuted-channel layout)
    pos = np.arange(S, dtype=np.float64)
    inv = THETA ** (-np.arange(0, DHR, 2, dtype=np.float64) / DHR)  # (32,)
    ang = inv[:, None] * pos[None, :]                               # (32, S)
    cosv = np.cos(ang).astype(f32)
    sinv = np.sin(ang).astype(f32)
    cs1 = np.ascontiguousarray(np.concatenate([cosv, cosv], axis=0))
    cs2 = np.ascontiguousarray(np.concatenate([-sinv, sinv], axis=0))

    # triangle mask for the 128-wide diagonal window: allow k <= q
    kk = np.arange(128)[:, None]
    qq = np.arange(128)[None, :]
    masktri = np.ascontiguousarray((kk <= qq).astype(f32))

    shared = {
        "xT0": np.ascontiguousarray(xT[:, 0:QB]),
        "wdqT": wdqT, "wdkvT": wdkvT, "wkrT": wkrT,
        "masktri": masktri, "cs1": cs1, "cs2": cs2,
        "ones128": np.ones((128, 128), bf16),
        "zeros64": np.zeros((DHR, S), bf16),
    }
    in_maps = []
    for c in range(N_CORES):
        cols = np.concatenate(
            [np.arange(qb * QB + c * SQB, qb * QB + (c + 1) * SQB)
             for qb in range(1, NQB)])
        hs = [c * HC + h for h in range(HC)]
        wuqT = np.concatenate(
            [W_UQ[h * DH:(h + 1) * DH, :].T for h in hs], axis=1)
        wqrT = np.concatenate(
            [W_QR[h * DHR:(h + 1) * DHR, :].T[:, perm] for h in hs], axis=1)
        wukT = np.concatenate(
            [W_UK[h * DH:(h + 1) * DH, :].T for h in hs], axis=1)
        wuvT = np.concatenate(
            [W_UV[h * DH:(h + 1) * DH, :].T for h in hs], axis=1)
        woT = np.concatenate(
            [W_O[:, h * DH:(h + 1) * DH].T for h in hs], axis=0)
        in_maps.append({
            **shared,
            "xTs": np.ascontiguousarray(xT[:, cols]),
            "csk1": np.ascontiguousarray(cs1[:, cols]),
            "csk2": np.ascontiguousarray(cs2[:, cols]),
            "wuqT": np.ascontiguousarray(wuqT).astype(bf16),
            "wqrT": np.ascontiguousarray(wqrT).astype(bf16),
            "wukT": np.ascontiguousarray(wukT).astype(bf16),
            "wuvT": np.ascontiguousarray(wuvT).astype(bf16),
            "woT": np.ascontiguousarray(woT).astype(bf16),
        })
    return in_maps


def kernel(**inputs):
    global LAST_EXEC_TIME_NS, LAST_RESULTS
    if "nc" not in _CACHE:
        _CACHE["nc"] = _build()
    nc = _CACHE["nc"]
    in_maps = _prep_inputs(**{k: np.asarray(v) for k, v in inputs.items()})
    kwargs = dict(TRACE_KWARGS)
    if TRACE:
        kwargs["trace"] = True
    res = run_bass_kernel_spmd(nc, in_maps, core_ids=list(range(N_CORES)),
                               **kwargs)
    LAST_EXEC_TIME_NS = res.exec_time_ns
    LAST_RESULTS = res
    acc = np.zeros((DM, S), np.float64)
    for c in range(N_CORES):
        acc += res.results[c]["outT"].astype(np.float64)
    return np.ascontiguousarray(acc.T[None]).astype(np.float32)


# revision 41
# speedup vs baseline: 1.1549x; 1.1549x over previous
"""Multi-Head Latent Attention (MLA) Trainium2 kernel, 8-core head-sharded,
with cross-core sharding of the latent down-projections.

v2 structural change vs baseline: stage 1 (c_Q / c_KV / k_R latents) was
replicated on all 8 cores (52% of PE cycles). Now:
  - query block 0's latents stay replicated (they are needed first, and
    the first collective on this fabric cannot complete before ~75us due
    to rank-launch skew - the replicated compute fills that window);
  - query blocks 1-3's latents are token-sharded 8 ways: each core
    computes all 9 latent chains for its own 64-token slice of each
    block (one fused 192-column matmul per (m-chunk, chain)), ropes its
    k_R slice locally, and three AllGathers (one per block, ~0.55MB wire
    per rank each) distribute the full latents while block-0/1 attention
    runs on the PE.
Everything downstream (per-head projections, attention, output
projection) is unchanged from the baseline except q_R: both heads'
rope-projections now run packed in one 128-wide stationary matmul.

Layout: all matmuls run with the contraction dim on partitions
("transposed world"); x and every weight are pre-transposed on the host.
Heads are sharded 2-per-core; each core emits a bf16 partial out.T (its
heads' contribution to the output projection), summed and transposed on
the host (rel err ~4e-3, harness gate 2e-2).

Precision: bf16 operands everywhere on the PE; all PSUM accumulation is
fp32. k_R/q_R are zero-padded to 128 partitions, and W_KR columns are
host-duplicated so the kr matmul has a full 128-wide stationary. The
softmax denominator is a matmul against an all-ones [128,128] stationary;
the reciprocal (reciprocal_approx_fast) is multiplied in directly.
"""
import sys

sys.path.insert(0, "/opt/trn_rl_repo")

import numpy as np

import concourse.bass as bass
import concourse.tile as tile
from concourse import bacc, mybir
from concourse.bass_utils import run_bass_kernel_spmd

F32 = mybir.dt.float32
BF16 = mybir.dt.bfloat16
AF = mybir.ActivationFunctionType
OP = mybir.AluOpType

N_CORES = 8
S = 2048          # sequence length
DM = 2048         # d_model
DL = 512          # d_latent
H = 16            # total heads
HC = H // N_CORES  # heads per core (2)
DH = 128          # head dim (content)
DHR = 64          # head dim (rope)
QB = 512          # query block
NQB = S // QB     # 4
KPB = QB // 128   # key chunks per query block (4)
NMC = DM // 128   # 16 model chunks
NLC = DL // 128   # 4 latent chunks
NKC = S // 128    # 16 key chunks
SQB = QB // N_CORES  # sharded tokens per (rank, block) = 64
NSH = NQB - 1     # sharded (gathered) query blocks: 1..3
LAT = 9 * 128     # gathered latent rows per token-slice (kr padded to 128)
THETA = 10000.0

SCALE = float(1.0 / np.sqrt(np.float32(DH + DHR)))
E_HI = float(np.exp(np.float64(80.0) * SCALE))
E_LO = float(np.exp(np.float64(-80.0) * SCALE))

# Set by test.py to profile; harness path leaves these untouched.
TRACE = False
TRACE_KWARGS = {}
LAST_EXEC_TIME_NS = None
LAST_RESULTS = None
DEBUG_DUMP = False

_CACHE = {}
MM_LABELS = {}


def _lbl(inst, label):
    try:
        MM_LABELS[inst.ins.name] = label
    except Exception:
        try:
            MM_LABELS[inst.name] = label
        except Exception:
            pass
    return inst


def _build():
    nc = bacc.Bacc("TRN2", target_bir_lowering=False, debug=False,
                   enable_asserts=True, num_devices=N_CORES)

    def din(name, shape, dt=BF16):
        return nc.dram_tensor(name, shape, dt, kind="ExternalInput").ap()

    d = {
        "xT0": din("xT0", [DM, QB]),              # block-0 tokens (replicated)
        "xTs": din("xTs", [DM, NSH * SQB]),       # this core's token slices
        "wdqT": din("wdqT", [DM, DL]),
        "wdkvT": din("wdkvT", [DM, DL]),
        "wkrT": din("wkrT", [DM, 128]),
        "wuqT": din("wuqT", [DL, HC * DH]),
        "wqrT": din("wqrT", [DL, HC * DHR]),
        "wukT": din("wukT", [DL, HC * DH]),
        "wuvT": din("wuvT", [DL, HC * DH]),
        "woT": din("woT", [HC * DH, DM]),
        "ones128": din("ones128", [128, 128]),
        "masktri": din("masktri", [128, 128], F32),
        "zeros64": din("zeros64", [64, S]),
        "cs1": din("cs1", [DHR, S], F32),
        "cs2": din("cs2", [DHR, S], F32),
        "csk1": din("csk1", [DHR, NSH * SQB], F32),
        "csk2": din("csk2", [DHR, NSH * SQB], F32),
        "outT": nc.dram_tensor("outT", [DM, S], BF16,
                               kind="ExternalOutput").ap(),
    }
    with tile.TileContext(nc) as tc:
        import contextlib
        with contextlib.ExitStack() as ctx:
            _kernel_body(ctx, tc, nc, d)
    nc.compile()
    return nc


def _kernel_body(ctx, tc, nc, d):
    wts = ctx.enter_context(tc.tile_pool(name="wts", bufs=1))
    kvp = ctx.enter_context(tc.tile_pool(name="kvp", bufs=1))
    xtp = ctx.enter_context(tc.tile_pool(name="xtp", bufs=1))
    lat = ctx.enter_context(tc.tile_pool(name="lat", bufs=1))
    prj = ctx.enter_context(tc.tile_pool(name="prj", bufs=1))
    smp = ctx.enter_context(tc.tile_pool(name="smp", bufs=1))
    stg = ctx.enter_context(tc.tile_pool(name="stg", bufs=1))
    o5p = ctx.enter_context(tc.tile_pool(name="o5p", bufs=16))
    dram = ctx.enter_context(tc.tile_pool(name="dram", bufs=1, space="DRAM"))

    # ---- stage-1 weights first: the sharded m-loop consumes
    # wkr[m]+wdq[m]+wdkv[m] per m, so issue them interleaved per m across
    # two queues ----
    wkr_t = [wts.tile([128, 128], BF16, name=f"wkr{m}") for m in range(NMC)]
    wdq_t = [wts.tile([128, DL], BF16, name=f"wdq{m}") for m in range(NMC)]
    wdkv_t = [wts.tile([128, DL], BF16, name=f"wdkv{m}") for m in range(NMC)]
    # pass-0 weights (wkr+wdq) for all m first, then pass-1 (wdkv): the
    # sharded m-loop never starves on late wdkv chunks
    for m in range(NMC):
        e = nc.gpsimd if m % 2 == 0 else nc.scalar
        e.dma_start(wkr_t[m][:], d["wkrT"][m * 128:(m + 1) * 128, :])
        e.dma_start(wdq_t[m][:], d["wdqT"][m * 128:(m + 1) * 128, :])
    for m in range(NMC):
        e = nc.gpsimd if m % 2 == 0 else nc.scalar
        e.dma_start(wdkv_t[m][:], d["wdkvT"][m * 128:(m + 1) * 128, :])

    # x for the sharded token slices (needed first), then block-0 x
    xts = [xtp.tile([128, NSH * SQB], BF16, name=f"xts{m}")
           for m in range(NMC)]
    for m in range(NMC):
        nc.sync.dma_start(xts[m][:], d["xTs"][m * 128:(m + 1) * 128, :])
    xt0 = [xtp.tile([128, QB], BF16, name=f"xt0_{m}") for m in range(NMC)]
    for m in range(NMC):
        nc.sync.dma_start(xt0[m][:], d["xT0"][m * 128:(m + 1) * 128, :])

    # small persistent loads
    o128_t = wts.tile([128, 128], BF16, name="o128")
    nc.scalar.dma_start(o128_t[:], d["ones128"][:, :])
    mask_t = wts.tile([128, 128], F32, name="masktri")
    nc.scalar.dma_start(mask_t[:], d["masktri"][:, :])
    csk1_t = smp.tile([DHR, NSH * SQB], F32, name="csk1t")
    csk2_t = smp.tile([DHR, NSH * SQB], F32, name="csk2t")
    nc.scalar.dma_start(csk1_t[:], d["csk1"][:, :])
    nc.scalar.dma_start(csk2_t[:], d["csk2"][:, :])
    wuq_t = [wts.tile([128, HC * DH], BF16, name=f"wuq{l}") for l in range(NLC)]
    wqr_t = [wts.tile([128, HC * DHR], BF16, name=f"wqr{l}") for l in range(NLC)]
    wuk_t = [wts.tile([128, HC * DH], BF16, name=f"wuk{l}") for l in range(NLC)]
    wuv_t = [wts.tile([128, HC * DH], BF16, name=f"wuv{l}") for l in range(NLC)]
    wo_t = [wts.tile([128, DM], BF16, name=f"wo{h}") for h in range(HC)]

    def emit_proj_dmas():
        for l in range(NLC):
            nc.gpsimd.dma_start(wuk_t[l][:], d["wukT"][l * 128:(l + 1) * 128, :])
            nc.gpsimd.dma_start(wuv_t[l][:], d["wuvT"][l * 128:(l + 1) * 128, :])
            nc.gpsimd.dma_start(wuq_t[l][:], d["wuqT"][l * 128:(l + 1) * 128, :])
            nc.gpsimd.dma_start(wqr_t[l][:], d["wqrT"][l * 128:(l + 1) * 128, :])

    def emit_wo_dmas():
        for h in range(HC):
            nc.gpsimd.dma_start(wo_t[h][:], d["woT"][h * 128:(h + 1) * 128, :])

    # ---- persistent per-sequence state ----
    kct = [kvp.tile([128, S], BF16, name=f"kct{h}") for h in range(HC)]
    # krt/qrt are zero-padded to 128 partitions: a 64-partition moving
    # operand runs matmuls at half rate.
    krt = kvp.tile([128, S], BF16, name="krt")
    nc.scalar.dma_start(krt[DHR:128, :], d["zeros64"][:, :])
    qrt = [kvp.tile([128, QB], BF16, name=f"qrt{h}") for h in range(HC)]
    for h in range(HC):
        nc.scalar.dma_start(qrt[h][DHR:128, :], d["zeros64"][:, 0:QB])
    vt = [kvp.tile([128, HC * DH], BF16, name=f"vt{k}") for k in range(NKC)]

    # ---- collective bounce buffers (internal DRAM) ----
    # Each block's gather is split: kv half (ckv chains + kr, 640 rows)
    # first, q half (cq chains, 512 rows) second - the k/v projections
    # can start as soon as the kv half lands.
    KVR = 5 * 128  # kv-half rows (ckv0-3 + kr)
    inb_kv = [dram.tile([KVR, SQB], BF16, name=f"inbkv{i}")
              for i in range(NSH)]
    inb_q = [dram.tile([DL, SQB], BF16, name=f"inbq{i}") for i in range(NSH)]
    outb_kv = [dram.tile([N_CORES * KVR, SQB], BF16, addr_space="Shared",
                         name=f"outbkv{i}") for i in range(NSH)]
    outb_q = [dram.tile([N_CORES * DL, SQB], BF16, addr_space="Shared",
                        name=f"outbq{i}") for i in range(NSH)]

    def rope(raw_pt, out_ap, cs1s, cs2s, tag, w=QB, swap_e=nc.sync):
        """raw_pt: PSUM [64, w] pre-rope rows; out_ap: bf16 dest [64, w].
        Swap-halves DMA reads PSUM directly. All ALU legs stay off
        gpsimd: after phase A that queue belongs to the collectives."""
        raw = smp.tile([DHR, w], F32, tag=f"rope_srcc{w}", name=f"rc_{tag}")
        nc.scalar.copy(raw[:], raw_pt)
        rsw = smp.tile([DHR, w], F32, tag=f"rope_swp{w}", name=f"rs_{tag}")
        swap_e.dma_start(rsw[0:32, :], raw[32:64, :])
        swap_e.dma_start(rsw[32:64, :], raw[0:32, :])
        rawm = smp.tile([DHR, w], F32, tag=f"rope_raw{w}", name=f"rr_{tag}")
        nc.vector.tensor_tensor(rawm[:], raw_pt, cs1s, op=OP.mult)
        nc.vector.tensor_tensor(rsw[:], rsw[:], cs2s, op=OP.mult)
        nc.vector.tensor_tensor(out_ap, rawm[:], rsw[:], op=OP.add)

    # ================= PHASE A: latents =================
    # chains: 0-3 cq[l], 4-7 ckv[l], 8 kr
    def sharded_stage1():
        """All 9 latent chains for this core's 64-token slices of blocks
        1..3, one fused 192-column matmul per (m, chain). Two passes of
        <=5 chains: interleaving two accumulation chains in one PSUM
        bank corrupts both, so every chain gets its own bank. Staged
        into a single SBUF tile so each block's gather input is ONE
        DMA."""
        W = NSH * SQB

        def st_for(j, m):
            if j < 4:
                return wdq_t[m][:, j * 128:(j + 1) * 128], "s1s_cq"
            if j < 8:
                return wdkv_t[m][:, (j - 4) * 128:(j - 3) * 128], "s1s_ckv"
            return wkr_t[m][:], "s1s_kr"

        ps = [tc.tile([128, W], F32, space="PSUM", name=f"pslat{t}")
              for t in range(5)]
        # staging layout: cq chains at cols 0:4W, ckv at 4W:8W, kr 8W:9W
        stall = stg.tile([128, 9 * W], BF16, name="stall")
        bank = {0: 0, 1: 1, 2: 2, 3: 3, 8: 4, 4: 0, 5: 1, 6: 2, 7: 3}
        for pa, chains in enumerate([(0, 1, 2, 3, 8), (4, 5, 6, 7)]):
            for m in range(NMC):
                for j in chains:
                    st_ap, label = st_for(j, m)
                    _lbl(nc.tensor.matmul(ps[bank[j]][0][:], st_ap,
                                          xts[m][:], start=(m == 0),
                                          stop=(m == NMC - 1)), label)
            for j in chains:
                if j == 8:
                    continue
                (nc.vector.tensor_copy if j % 2 == 0 else nc.scalar.copy)(
                    stall[:, j * W:(j + 1) * W], ps[bank[j]][0][:])
            if pa == 0:
                krp = ps[4][0][0:DHR, :]
                rope(krp, stall[0:DHR, 8 * W:9 * W], csk1_t[:], csk2_t[:],
                     "krs", w=W, swap_e=nc.scalar)
                # kr slot partitions 64-127 are never computed; zero them
                # so the gather doesn't ship uninitialized SBUF
                nc.scalar.copy(stall[DHR:128, 8 * W:9 * W], krt[DHR:128, 0:W])
        for i in range(NSH):
            isl = slice(i * SQB, (i + 1) * SQB)
            src_kv = stall[:, 4 * W:9 * W].rearrange(
                "p (j w) -> p j w", w=W)[:, :, isl]
            nc.gpsimd.dma_start(
                inb_kv[i][:].rearrange("(j p) t -> p j t", p=128), src_kv)
            src_q = stall[:, 0:4 * W].rearrange(
                "p (j w) -> p j w", w=W)[:, :, isl]
            nc.gpsimd.dma_start(
                inb_q[i][:].rearrange("(j p) t -> p j t", p=128), src_q)
        return ps

    def issue_ags():
        for i in range(NSH):
            nc.gpsimd.collective_compute(
                "AllGather", mybir.AluOpType.bypass,
                replica_groups=[list(range(N_CORES))],
                ins=[inb_kv[i].opt()], outs=[outb_kv[i].opt()])
            nc.gpsimd.collective_compute(
                "AllGather", mybir.AluOpType.bypass,
                replica_groups=[list(range(N_CORES))],
                ins=[inb_q[i].opt()], outs=[outb_q[i].opt()])

    srot = [0]

    def rep_stage1_qb0():
        """Block-0 latents replicated on every core (baseline stage-1)."""
        cs1s = smp.tile([DHR, QB], F32, tag="cs1s", bufs=2, name="cs1s0")
        cs2s = smp.tile([DHR, QB], F32, tag="cs2s", bufs=2, name="cs2s0")
        nc.scalar.dma_start(cs1s[:], d["cs1"][:, 0:QB])
        nc.scalar.dma_start(cs2s[:], d["cs2"][:, 0:QB])
        ckv = [lat.tile([128, QB], BF16, tag=f"ckv{l}", bufs=2,
                        name=f"ckv{l}_0") for l in range(NLC)]
        cq = [lat.tile([128, QB], BF16, tag=f"cq{l}", bufs=2,
                       name=f"cq{l}_0") for l in range(NLC)]
        eng_tgl = [0]

        def copy_out(dst, src):
            (nc.vector.tensor_copy if eng_tgl[0] % 2 == 0
             else nc.scalar.copy)(dst, src)
            eng_tgl[0] += 1

        plan = [
            [("kr", None), ("cq", 0), ("cq", 1)],
            [("cq", 2), ("cq", 3)],
            [("ckv", 0), ("ckv", 1)],
            [("ckv", 2), ("ckv", 3)],
        ]
        # 3 rotating PSUM banks across the passes (5 banks hold the
        # sharded chains until the gather inputs are staged)
        rep_ps = [tc.tile([128, QB], F32, space="PSUM", name=f"repps{i}")
                  for i in range(3)]
        rot = [0]
        for pi, groups in enumerate(plan):
            pts = []
            for gi, (kind, idx) in enumerate(groups):
                pts.append(rep_ps[rot[0] % 3][0])
                rot[0] += 1
            for m in range(NMC):
                for gi, (kind, idx) in enumerate(groups):
                    if kind == "kr":
                        st_ap, label = wkr_t[m][:], "s1_kr"
                    elif kind == "cq":
                        st_ap = wdq_t[m][:, idx * 128:(idx + 1) * 128]
                        label = "s1_cq"
                    else:
                        st_ap = wdkv_t[m][:, idx * 128:(idx + 1) * 128]
                        label = "s1_ckv"
                    _lbl(nc.tensor.matmul(pts[gi][:], st_ap, xt0[m][:],
                                          start=(m == 0),
                                          stop=(m == NMC - 1)), label)
            for gi, (kind, idx) in enumerate(groups):
                if kind == "kr":
                    rope(pts[gi][0:DHR, :], krt[0:DHR, 0:QB], cs1s[:],
                         cs2s[:], "kr0", swap_e=nc.scalar)
                elif kind == "cq":
                    copy_out(cq[idx][:], pts[gi][:])
                else:
                    copy_out(ckv[idx][:], pts[gi][:])
            if pi == 0:
                emit_proj_dmas()
                emit_wo_dmas()
        for t, free in reversed(rep_ps):
            free()
        return cq, ckv, cs1s, cs2s

    pslat = sharded_stage1()
    issue_ags()
    cq0, ckv0, cs1s0, cs2s0 = rep_stage1_qb0()
    for t, free in reversed(pslat):
        free()

    # ================= PHASE B: attention =================
    ps_s1 = ctx.enter_context(tc.tile_pool(name="ps_s1", bufs=1, space="PSUM"))
    ps_at = ctx.enter_context(tc.tile_pool(name="ps_at", bufs=2, space="PSUM"))
    ps_sm = ctx.enter_context(tc.tile_pool(name="ps_sm", bufs=2, space="PSUM"))

    s1rot = [0]

    def s1tile(shape, name):
        t = ps_s1.tile(shape, F32, tag=f"s1{s1rot[0] % 4}", name=name)
        s1rot[0] += 1
        return t

    def dma_back(qb):
        """Load block qb's gathered latents from the AllGather output.
        One 3D-access-pattern DMA per SBUF tile (9 per block): rank r's
        [128, 64] sub-block lands at columns r*64."""
        i = qb - 1
        ckv = [lat.tile([128, QB], BF16, tag=f"ckv{l}", bufs=2,
                        name=f"ckv{l}_{qb}") for l in range(NLC)]
        cq = [lat.tile([128, QB], BF16, tag=f"cq{l}", bufs=2,
                       name=f"cq{l}_{qb}") for l in range(NLC)]
        gkv = outb_kv[i][:].rearrange("(r q) t -> r q t", r=N_CORES)
        for l in range(NLC):
            nc.sync.dma_start(
                ckv[l][:].rearrange("p (r t) -> p r t", r=N_CORES),
                gkv[:, l * 128:(l + 1) * 128, :].transpose([1, 0, 2]))
        nc.sync.dma_start(
            krt[0:DHR, qb * QB:(qb + 1) * QB].rearrange(
                "p (r t) -> p r t", r=N_CORES),
            gkv[:, 4 * 128:4 * 128 + DHR, :].transpose([1, 0, 2]))
        gq = outb_q[i][:].rearrange("(r q) t -> r q t", r=N_CORES)
        for l in range(NLC):
            nc.sync.dma_start(
                cq[l][:].rearrange("p (r t) -> p r t", r=N_CORES),
                gq[:, l * 128:(l + 1) * 128, :].transpose([1, 0, 2]))
        cs1s = smp.tile([DHR, QB], F32, tag="cs1s", bufs=2, name=f"cs1s{qb}")
        cs2s = smp.tile([DHR, QB], F32, tag="cs2s", bufs=2, name=f"cs2s{qb}")
        qsl = slice(qb * QB, (qb + 1) * QB)
        nc.scalar.dma_start(cs1s[:], d["cs1"][:, qsl])
        nc.scalar.dma_start(cs2s[:], d["cs2"][:, qsl])
        return cq, ckv, cs1s, cs2s

    def stage2(qb, cq, ckv, cs1s, cs2s):
        # kv half first: ckv arrives one collective earlier than cq
        qsl = slice(qb * QB, (qb + 1) * QB)
        for h in range(HC):
            pkc = s1tile([128, QB], f"pkc{h}_{qb}")
            for l in range(NLC):
                _lbl(nc.tensor.matmul(pkc[:], wuk_t[l][:, h * DH:(h + 1) * DH],
                                      ckv[l][:], start=(l == 0),
                                      stop=(l == NLC - 1)), "s2_kc")
            (nc.vector.tensor_copy if h == 0 else nc.scalar.copy)(
                kct[h][:, qsl], pkc[:])
        for sc in range(KPB):
            k = qb * KPB + sc
            pv = s1tile([128, HC * DH], f"pvv{k}")
            for l in range(NLC):
                _lbl(nc.tensor.matmul(pv[:], ckv[l][:, sc * 128:(sc + 1) * 128],
                                      wuv_t[l][:], start=(l == 0),
                                      stop=(l == NLC - 1)), "s2_v")
            (nc.vector.tensor_copy if sc % 2 == 0 else nc.scalar.copy)(
                vt[k][:], pv[:])
        # q_R both heads packed in one 128-wide stationary chain
        pqr = s1tile([128, QB], f"pqr_{qb}")
        for l in range(NLC):
            _lbl(nc.tensor.matmul(pqr[:], wqr_t[l][:], cq[l][:],
                                  start=(l == 0), stop=(l == NLC - 1)),
                 "s2_qr")
        for h in range(HC):
            # swap legs on scalar: the sync queue is busy with the
            # gather-readback DMAs right when this rope runs
            rope(pqr[h * DHR:(h + 1) * DHR, :], qrt[h][0:DHR, :],
                 cs1s[:], cs2s[:], f"qr{h}_{qb}", swap_e=nc.scalar)
        # q_C per head
        qct = [prj.tile([128, QB], BF16, tag=f"qct{h}", bufs=2,
                        name=f"qct{h}_{qb}") for h in range(HC)]
        for h in range(HC):
            pqc = s1tile([128, QB], f"pqc{h}_{qb}")
            for l in range(NLC):
                _lbl(nc.tensor.matmul(pqc[:], wuq_t[l][:, h * DH:(h + 1) * DH],
                                      cq[l][:], start=(l == 0),
                                      stop=(l == NLC - 1)), "s2_qc")
            nc.vector.tensor_copy(qct[h][:], pqc[:])
        return qct

    def attn_both(qb, qct, qrt):
        """Both heads interleaved per key chunk: 2x PE density per chain
        step."""
        nkc = KPB * (qb + 1)
        pat = [ps_at.tile([128, QB], F32, tag="at", name=f"pat{h}_{qb}")
               for h in range(HC)]
        psums = [ps_sm.tile([128, QB], F32, tag="smrb", name=f"psums{h}_{qb}")
                 for h in range(HC)]
        pend = []  # (h, kc, off, pt) awaiting PV+sums

        def flush(last):
            h, kc, off, pt = pend.pop(0)
            _lbl(nc.tensor.matmul(psums[h][:, off:], o128_t[:], pt[:, off:],
                                  start=(kc == 0), stop=last,
                                  skip_group_check=True), "sum")
            _lbl(nc.tensor.matmul(pat[h][:, off:],
                                  vt[kc][:, h * DH:(h + 1) * DH],
                                  pt[:, off:], start=(kc == 0), stop=last,
                                  skip_group_check=True), "pv")

        for kc in range(nkc):
            off = 128 * (kc - KPB * qb) if kc >= KPB * qb else 0
            w = QB - off
            ksl = slice(kc * 128, (kc + 1) * 128)
            for h in range(HC):
                ps_s = s1tile([128, QB], f"s{h}_{qb}_{kc}")
                _lbl(nc.tensor.matmul(ps_s[:, off:], kct[h][:, ksl],
                                      qct[h][:, off:], start=True, stop=False,
                                      skip_group_check=True), "qk_c")
                _lbl(nc.tensor.matmul(ps_s[:, off:], krt[:, ksl],
                                      qrt[h][:, off:], start=False, stop=True,
                                      skip_group_check=True), "qk_r")
                if len(pend) >= 2:
                    flush(False)
                et = smp.tile([128, QB], BF16, tag="et", bufs=5,
                              name=f"et{h}_{qb}_{kc}")
                nc.scalar.activation(et[:, off:], ps_s[:, off:], AF.Exp,
                                     scale=SCALE)
                pt = smp.tile([128, QB], BF16, tag="pt", bufs=5,
                              name=f"pt{h}_{qb}_{kc}")
                if kc >= KPB * qb:  # diagonal: clip+mask window, clip rest
                    ctw = smp.tile([128, 128], BF16, tag="ctw", bufs=3,
                                   name=f"ctw{h}_{qb}_{kc}")
                    nc.vector.tensor_scalar(ctw[:], et[:, off:off + 128],
                                            E_HI, E_LO, op0=OP.min, op1=OP.max)
                    nc.vector.tensor_tensor(pt[:, off:off + 128], ctw[:],
                                            mask_t[:], op=OP.mult)
                    if w > 128:
                        nc.vector.tensor_scalar(pt[:, off + 128:],
                                                et[:, off + 128:], E_HI, E_LO,
                                                op0=OP.min, op1=OP.max)
                else:
                    nc.vector.tensor_scalar(pt[:], et[:], E_HI, E_LO,
                                            op0=OP.min, op1=OP.max)
                pend.append((h, kc, off, pt))
        while len(pend) > 2:
            flush(False)
        while pend:
            flush(True)
        return pat, psums

    def attn_sum(qb, h, psums):  # psums: [128,QB], denom bcast on partitions
        rcr = smp.tile([128, QB], F32, tag="rcr", bufs=2, name=f"rcr{h}_{qb}")
        nc.vector.reciprocal_approx_fast(rcr[:], psums[:])
        return rcr

    def attn_norm(qb, h, pat, rcr, attn_n):
        nc.vector.tensor_tensor(attn_n[:], pat[:], rcr[:], op=OP.mult)

    def stage5(qb, attn_n):
        qsl = slice(qb * QB, (qb + 1) * QB)
        for m in range(NMC):
            po = s1tile([128, QB], f"po{m}_{qb}")
            for h in range(HC):
                _lbl(nc.tensor.matmul(po[:], wo_t[h][:, m * 128:(m + 1) * 128],
                                      attn_n[h][:], start=(h == 0),
                                      stop=(h == HC - 1)), "s5")
            ob = o5p.tile([128, QB], BF16, tag="ob", name=f"ob{m}_{qb}")
            (nc.vector.tensor_copy if m % 2 == 0 else nc.scalar.copy)(
                ob[:], po[:])
            nc.gpsimd.dma_start(d["outT"][m * 128:(m + 1) * 128, qsl], ob[:])

    # ---- main loop ----
    cq, ckv, cs1s, cs2s = cq0, ckv0, cs1s0, cs2s0
    qct = stage2(0, cq, ckv, cs1s, cs2s)
    for qb in range(NQB):
        attn_n = [prj.tile([128, QB], BF16, tag=f"an{h}", name=f"an{h}_{qb}")
                  for h in range(HC)]
        pat, psums = attn_both(qb, qct, qrt)
        rcr0 = attn_sum(qb, 0, psums[0])
        rcr1 = attn_sum(qb, 1, psums[1])
        attn_norm(qb, 0, pat[0], rcr0, attn_n[0][:])
        attn_norm(qb, 1, pat[1], rcr1, attn_n[1][:])
        if qb < NQB - 1:
            cq, ckv, cs1s, cs2s = dma_back(qb + 1)
            qct = stage2(qb + 1, cq, ckv, cs1s, cs2s)
        stage5(qb, attn_n)


def _prep_inputs(x, W_DQ, W_UQ, W_QR, W_DKV, W_UK, W_UV, W_KR, W_O):
    """Host-side sharding + layout prep. Returns list of 8 in_maps."""
    import ml_dtypes
    f32 = np.float32
    bf16 = ml_dtypes.bfloat16
    xT = np.ascontiguousarray(x[0].T).astype(bf16)
    perm = np.concatenate([np.arange(0, DHR, 2), np.arange(1, DHR, 2)])
    wdqT = np.ascontiguousarray(W_DQ.T).astype(bf16)
    wdkvT = np.ascontiguousarray(W_DKV.T).astype(bf16)
    wkrT = np.ascontiguousarray(
        np.concatenate([W_KR.T[:, perm]] * 2, axis=1)).astype(bf16)

    # rope tables (transposed, permuted-channel layout)
    pos = np.arange(S, dtype=np.float64)
    inv = THETA ** (-np.arange(0, DHR, 2, dtype=np.float64) / DHR)  # (32,)
    ang = inv[:, None] * pos[None, :]                               # (32, S)
    cosv = np.cos(ang).astype(f32)
    sinv = np.sin(ang).astype(f32)
    cs1 = np.ascontiguousarray(np.concatenate([cosv, cosv], axis=0))
    cs2 = np.ascontiguousarray(np.concatenate([-sinv, sinv], axis=0))

    # triangle mask for the 128-wide diagonal window: allow k <= q
    kk = np.arange(128)[:, None]
    qq = np.arange(128)[None, :]
    masktri = np.ascontiguousarray((kk <= qq).astype(f32))

    shared = {
        "xT0": np.ascontiguousarray(xT[:, 0:QB]),
        "wdqT": wdqT, "wdkvT": wdkvT, "wkrT": wkrT,
        "masktri": masktri, "cs1": cs1, "cs2": cs2,
        "ones128": np.ones((128, 128), bf16),
        "zeros64": np.zeros((DHR, S), bf16),
    }
    in_maps = []
    for c in range(N_CORES):
        cols = np.concatenate(
            [np.arange(qb * QB + c * SQB, qb * QB + (c + 1) * SQB)
             for qb in range(1, NQB)])
        hs = [c * HC + h for h in range(HC)]
        wuqT = np.concatenate(
            [W_UQ[h * DH:(h + 1) * DH, :].T for h in hs], axis=1)
        wqrT = np.concatenate(
            [W_QR[h * DHR:(h + 1) * DHR, :].T[:, perm] for h in hs], axis=1)
        wukT = np.concatenate(
            [W_UK[h * DH:(h + 1) * DH, :].T for h in hs], axis=1)
        wuvT = np.concatenate(
            [W_UV[h * DH:(h + 1) * DH, :].T for h in hs], axis=1)
        woT = np.concatenate(
            [W_O[:, h * DH:(h + 1) * DH].T for h in hs], axis=0)
        in_maps.append({
            **shared,
            "xTs": np.ascontiguousarray(xT[:, cols]),
            "csk1": np.ascontiguousarray(cs1[:, cols]),
            "csk2": np.ascontiguousarray(cs2[:, cols]),
            "wuqT": np.ascontiguousarray(wuqT).astype(bf16),
            "wqrT": np.ascontiguousarray(wqrT).astype(bf16),
            "wukT": np.ascontiguousarray(wukT).astype(bf16),
            "wuvT": np.ascontiguousarray(wuvT).astype(bf16),
            "woT": np.ascontiguousarray(woT).astype(bf16),
        })
    return in_maps


def kernel(**inputs):
    global LAST_EXEC_TIME_NS, LAST_RESULTS
    if "nc" not in _CACHE:
        _CACHE["nc"] = _build()
    nc = _CACHE["nc"]
    in_maps = _prep_inputs(**{k: np.asarray(v) for k, v in inputs.items()})
    kwargs = dict(TRACE_KWARGS)
    if TRACE:
        kwargs["trace"] = True
    res = run_bass_kernel_spmd(nc, in_maps, core_ids=list(range(N_CORES)),
                               **kwargs)
    LAST_EXEC_TIME_NS = res.exec_time_ns
    LAST_RESULTS = res
    acc = np.zeros((DM, S), np.float64)
    for c in range(N_CORES):
        acc += res.results[c]["outT"].astype(np.float64)
    return np.ascontiguousarray(acc.T[None]).astype(np.float32)
